# revision 4
# baseline (speedup 1.0000x reference)
"""GQA attention block (q 32 heads / kv 8 heads, T=2048, C=4096) on 8 trn2
NeuronCores.

Sharding: tensor-parallel over heads x data-parallel over batch.
Core c handles batch b = c//4 and head-group g = c%4 (8 q heads, 2 kv heads).
Each core computes q/k/v projections for its head slice, RoPE, causal
flash-attention, and a row-parallel slice of the output projection; the host
sums the 4 partial outputs per batch.

Device-side layouts keep the head dim (hs) on SBUF partitions:
  Q^T, K^T: [hs=128, tok]  (projections emit transposed directly;
                            rotate-half is a 128x128 matmul by a constant
                            permutation matrix R, so no cross-partition DVE)
  V:        [tok, hs]      (projected as V^T, then PE-transposed)
  scores^T: [tk, tq]       (softmax denominators via ones-matmul)
Matmuls run in float32r (fp32 storage, ~1.4e-4 rounding, bf16-rate at
free-dim >= 256).
"""

import os
import sys

for _p in ("/root/.axon_site", "/root/.axon_site/_ro/trn_rl_repo",
           "/root/.axon_site/_ro/pypackages", "/opt/trn_rl_repo", "/opt/pypackages"):
    if os.path.isdir(_p) and _p not in sys.path:
        sys.path.append(_p)

import numpy as np

import concourse.bass as bass
import concourse.tile as tile
from concourse import mybir
from concourse.bass_utils import run_bass_kernel_spmd

F32 = mybir.dt.float32
F32R = mybir.dt.float32r
BF16 = mybir.dt.bfloat16

B, T, C = 2, 2048, 4096
H, KVH, HS = 32, 8, 128
ROPE_BASE = 10000.0

NCORES = 8
TPG = 4               # tensor-parallel groups per batch
HL = H // TPG         # 8 local q heads
KVL = KVH // TPG      # 2 local kv heads
GQ = H // KVH         # 4 q heads per kv head
CCH = C // 128        # 32 contraction chunks
PBLK = 256            # proj token block (free dim)
NPB = T // PBLK       # 8
TQB = 512             # attention tq block
NTQ = T // TQB        # 4
NTK = T // 128        # 16 tk chunks
SCALE = float(1.0 / np.sqrt(HS))

DT = F32R             # matmul operand dtype


def _split1(nc, max_waits=1):
    """Split instructions with >1 sem wait into preceding NOPs (the cayman
    CTRL codegen only accepts one sync-wait command per instruction)."""
    n = 0
    for f in nc.m.functions:
        for bb in f.blocks:
            out = []
            for inst in bb.instructions:
                si = inst.sync_info
                if si is not None and si.on_wait and len(si.on_wait) > max_waits:
                    w = list(si.on_wait)
                    chunks = [w[i:i + max_waits] for i in range(0, len(w), max_waits)]
                    for j, ch in enumerate(chunks[:-1]):
                        out.append(mybir.InstNoOp(
                            name=f"{inst.name}-wsplit{j}", engine=inst.engine,
                            ins=[], outs=[],
                            sync_info=mybir.SyncInfo(on_wait=ch, on_update=[])))
                        n += 1
                    inst.sync_info = mybir.SyncInfo(
                        on_wait=chunks[-1], on_update=list(si.on_update))
                out.append(inst)
            bb.instructions[:] = out
    return n


def build_nc(with_bias=False, split=True):
    nc = bass.Bass("TRN2")
    P = 128

    # --- DRAM parameters (per-core shards, host-pre-tiled layouts) ---
    dp = nc.declare_dram_parameter
    xq_d = dp("xq", [P, NPB, CCH, PBLK], DT, isOutput=False)    # [ki, blk, ko, tw]
    xkv_d = dp("xkv", [P, NPB, CCH, PBLK], DT, isOutput=False)
    wq_d = dp("wq", [P, CCH, HL * HS], DT, isOutput=False)      # [ki, ko, n]
    wkv_d = dp("wkv", [P, CCH, 2 * KVL * HS], DT, isOutput=False)  # [K2 | V2]
    wo_d = dp("wo", [P, HL * HS // P, C], DT, isOutput=False)
    cosT_d = dp("cosT", [HS, T], F32, isOutput=False)
    sinT_d = dp("sinT", [HS, T], F32, isOutput=False)
    rt_d = dp("rt", [P, P], DT, isOutput=False)                 # R^T rotate-half
    ident_d = dp("ident", [P, P], DT, isOutput=False)
    ones_d = dp("ones", [P, P], DT, isOutput=False)
    mask_d = dp("mask", [P, 2 * TQB], DT, isOutput=False)       # mult. causal
    if with_bias:
        bq_d = dp("bq", [HL * HS], F32, isOutput=False)
        bkv_d = dp("bkv", [2 * KVL * HS], F32, isOutput=False)
    out_d = dp("out", [T, C], F32, isOutput=True)

    qT_d = nc.dram_tensor("qT_i", [HS, HL, T], DT)              # internal

    with tile.TileContext(nc) as tc:
        with (
            tc.tile_pool(name="consts", bufs=1) as consts,
            tc.tile_pool(name="kvres", bufs=1) as kvres,
        ):
            rt_sb = consts.tile([P, P], DT)
            ident_sb = consts.tile([P, P], DT)
            ones_sb = consts.tile([P, P], DT)
            mask_sb = consts.tile([P, 2 * TQB], DT)
            nc.sync.dma_start(out=rt_sb, in_=rt_d[:])
            nc.sync.dma_start(out=ident_sb, in_=ident_d[:])
            nc.sync.dma_start(out=ones_sb, in_=ones_d[:])
            nc.sync.dma_start(out=mask_sb, in_=mask_d[:])
            if with_bias:
                bq_sb = consts.tile([P, HL], F32)
                bkv_sb = consts.tile([P, 2 * KVL], F32)
                nc.sync.dma_start(out=bq_sb, in_=bq_d[:].rearrange("(h p) -> p h", p=P))
                nc.sync.dma_start(out=bkv_sb, in_=bkv_d[:].rearrange("(h p) -> p h", p=P))

            kT_sb = kvres.tile([HS, KVL, T], DT)
            v_sb = kvres.tile([P, NTK, KVL * HS], DT)

            # ---------- P1 + P2: projections + RoPE ----------
            with (
                tc.tile_pool(name="wpool", bufs=1) as wpool,
                tc.tile_pool(name="xpool", bufs=2) as xpool,
                tc.tile_pool(name="tblp", bufs=2) as tblp,
                tc.tile_pool(name="ptmp", bufs=3) as ptmp,
                tc.tile_pool(name="stage", bufs=3) as stage,
                tc.tile_pool(name="pp_proj", bufs=2, space="PSUM") as pp_proj,
                tc.tile_pool(name="pp_rot", bufs=2, space="PSUM") as pp_rot,
                tc.tile_pool(name="pp_t", bufs=2, space="PSUM") as pp_t,
            ):
                def rope(dst, raw, rot_ps, blk, tmp_pool):
                    """dst = raw*cosT + rot_ps*sinT over a [128, PBLK] block."""
                    cos_t = tblp.tile([HS, PBLK], F32, tag="cos")
                    sin_t = tblp.tile([HS, PBLK], F32, tag="sin")
                    sl = slice(blk * PBLK, (blk + 1) * PBLK)
                    nc.sync.dma_start(out=cos_t, in_=cosT_d[:, sl])
                    nc.sync.dma_start(out=sin_t, in_=sinT_d[:, sl])
                    tmp = tmp_pool.tile([HS, PBLK], F32, tag="ropetmp")
                    nc.vector.tensor_mul(tmp, rot_ps, sin_t)
                    nc.vector.tensor_mul(dst, raw, cos_t)
                    nc.vector.tensor_add(dst, dst, tmp)

                # P1: K^T/V^T projections (+ V transpose to natural layout)
                wkv_t = wpool.tile([P, CCH, 2 * KVL * HS], DT, tag="w")
                nc.sync.dma_start(out=wkv_t, in_=wkv_d[:])
                for blk in range(NPB):
                    xkv_t = xpool.tile([P, CCH, PBLK], DT, tag="x")
                    nc.sync.dma_start(out=xkv_t, in_=xkv_d[:, blk])
                    for ch in range(2 * KVL):          # k0,k1,v0,v1
                        ps = pp_proj.tile([P, PBLK], F32, tag="proj")
                        for c in range(CCH):
                            nc.tensor.matmul(
                                ps, lhsT=wkv_t[:, c, ch * P:(ch + 1) * P],
                                rhs=xkv_t[:, c, :],
                                start=(c == 0), stop=(c == CCH - 1))
                        raw = ptmp.tile([P, PBLK], DT, tag="raw")
                        if with_bias:
                            nc.vector.tensor_scalar(
                                out=raw, in0=ps, scalar1=bkv_sb[:, ch:ch + 1],
                                scalar2=None, op0=mybir.AluOpType.add)
                        else:
                            nc.any.tensor_copy(out=raw, in_=ps)
                        if ch < KVL:                   # K head: RoPE -> kT_sb
                            rot = pp_rot.tile([P, PBLK], F32, tag="rot")
                            nc.tensor.matmul(rot, lhsT=rt_sb, rhs=raw,
                                             start=True, stop=True)
                            dst = kT_sb[:, ch, blk * PBLK:(blk + 1) * PBLK]
                            rope(dst, raw, rot, blk, ptmp)
                        else:                          # V head: transpose
                            kv = ch - KVL
                            for sub in range(PBLK // P):
                                pt = pp_t.tile([P, P], DT, tag="tp")
                                nc.tensor.transpose(
                                    pt, raw[:, sub * P:(sub + 1) * P], ident_sb)
                                nc.any.tensor_copy(
                                    out=v_sb[:, 2 * blk + sub,
                                             kv * HS:(kv + 1) * HS],
                                    in_=pt)

                # P2: Q^T projection (2 weight halves)
                for half in range(2):
                    wq_t = wpool.tile([P, CCH, HL * HS // 2], DT, tag="w")
                    nc.sync.dma_start(
                        out=wq_t, in_=wq_d[:, :, half * 512:(half + 1) * 512])
                    for blk in range(NPB):
                        xq_t = xpool.tile([P, CCH, PBLK], DT, tag="x")
                        nc.sync.dma_start(out=xq_t, in_=xq_d[:, blk])
                        for hb in range(HL // 2):
                            h = half * (HL // 2) + hb
                            ps = pp_proj.tile([P, PBLK], F32, tag="proj")
                            for c in range(CCH):
                                nc.tensor.matmul(
                                    ps, lhsT=wq_t[:, c, hb * P:(hb + 1) * P],
                                    rhs=xq_t[:, c, :],
                                    start=(c == 0), stop=(c == CCH - 1))
                            raw = ptmp.tile([P, PBLK], DT, tag="raw")
                            if with_bias:
                                nc.vector.tensor_scalar(
                                    out=raw, in0=ps, scalar1=bq_sb[:, h:h + 1],
                                    scalar2=None, op0=mybir.AluOpType.add)
                            else:
                                nc.any.tensor_copy(out=raw, in_=ps)
                            rot = pp_rot.tile([P, PBLK], F32, tag="rot")
                            nc.tensor.matmul(rot, lhsT=rt_sb, rhs=raw,
                                             start=True, stop=True)
                            qro = stage.tile([P, PBLK], DT, tag="qro")
                            rope(qro, raw, rot, blk, ptmp)
                            nc.sync.dma_start(
                                out=qT_d[:, h, blk * PBLK:(blk + 1) * PBLK],
                                in_=qro)

            # ---------- P3: attention, P4: output projection ----------
            with (
                tc.tile_pool(name="ybuf", bufs=1) as ybuf,
                tc.tile_pool(name="qbuf", bufs=2) as qbuf,
                tc.tile_pool(name="pbuf", bufs=3) as pbuf,
                tc.tile_pool(name="rbuf", bufs=2) as rbuf,
                tc.tile_pool(name="pp_s", bufs=2, space="PSUM") as pp_s,
                tc.tile_pool(name="pp_acc", bufs=2, space="PSUM") as pp_acc,
            ):
                yT_sb = ybuf.tile([HS, HL, T], DT)
                for h in range(HL):
                    kv = h // GQ
                    for j in range(NTQ):
                        qb = qbuf.tile([HS, TQB], DT, tag="qb")
                        nc.sync.dma_start(
                            out=qb, in_=qT_d[:, h, j * TQB:(j + 1) * TQB])
                        ps_y = pp_acc.tile([P, TQB], F32, tag="accy")
                        ps_sum = pp_acc.tile([P, TQB], F32, tag="accs")
                        nchunks = (j + 1) * (TQB // P)
                        for a in range(nchunks):
                            ps_s = pp_s.tile([P, TQB], F32, tag="s")
                            nc.tensor.matmul(
                                ps_s, lhsT=kT_sb[:, kv, a * P:(a + 1) * P],
                                rhs=qb, start=True, stop=True)
                            pT = pbuf.tile([P, TQB], DT, tag="pT")
                            nc.scalar.activation(
                                out=pT, in_=ps_s,
                                func=mybir.ActivationFunctionType.Exp,
                                scale=SCALE)
                            m = a - j * (TQB // P)
                            if m >= 0:                 # diagonal chunk
                                nc.vector.tensor_mul(
                                    pT, pT,
                                    mask_sb[:, TQB - m * P: 2 * TQB - m * P])
                            nc.tensor.matmul(
                                ps_y, lhsT=v_sb[:, a, kv * HS:(kv + 1) * HS],
                                rhs=pT, start=(a == 0), stop=(a == nchunks - 1))
                            nc.tensor.matmul(
                                ps_sum, lhsT=ones_sb, rhs=pT,
                                start=(a == 0), stop=(a == nchunks - 1))
                        rec = rbuf.tile([P, TQB], F32, tag="rec")
                        nc.vector.reciprocal(out=rec, in_=ps_sum)
                        nc.vector.tensor_mul(
                            yT_sb[:, h, j * TQB:(j + 1) * TQB], ps_y, rec)

                # P4: out = yT.T @ Wo (row-parallel partial)
                with (
                    tc.tile_pool(name="wobuf", bufs=2) as wobuf,
                    tc.tile_pool(name="obuf", bufs=3) as obuf,
                    tc.tile_pool(name="pp_o", bufs=2, space="PSUM") as pp_o,
                ):
                    NCH = HL * HS // P                 # 8
                    for o in range(C // TQB):          # 8 output chunks
                        wo_t = wobuf.tile([P, NCH, TQB], DT, tag="wo")
                        nc.sync.dma_start(
                            out=wo_t, in_=wo_d[:, :, o * TQB:(o + 1) * TQB])
                        for i in range(NTK):           # 16 token chunks
                            ps = pp_o.tile([P, TQB], F32, tag="o")
                            for chl in range(NCH):
                                nc.tensor.matmul(
                                    ps, lhsT=yT_sb[:, chl, i * P:(i + 1) * P],
                                    rhs=wo_t[:, chl, :],
                                    start=(chl == 0), stop=(chl == NCH - 1))
                            ot = obuf.tile([P, TQB], F32, tag="ot")
                            nc.any.tensor_copy(out=ot, in_=ps)
                            nc.sync.dma_start(
                                out=out_d[i * P:(i + 1) * P,
                                          o * TQB:(o + 1) * TQB],
                                in_=ot)

    if split:
        _split1(nc)
    return nc


def _rope_tables():
    inv_freq = (1.0 / (np.float32(ROPE_BASE) **
                       (np.arange(0, HS, 2, dtype=np.float32) / np.float32(HS))))
    pos = np.arange(T, dtype=np.float32)
    ang = pos[:, None] * inv_freq[None, :]
    ang = np.concatenate([ang, ang], axis=-1).astype(np.float32)  # [T, HS]
    return np.cos(ang).astype(np.float32), np.sin(ang).astype(np.float32)


def _tile_x(x):
    # [T, C] -> [ki=128, blk, ko, tw]  (x^T tiled for contraction-major DMA)
    return np.ascontiguousarray(
        x.reshape(NPB, PBLK, CCH, 128).transpose(3, 0, 2, 1))


def _tile_w(w):
    # [C, N] -> [ki=128, ko, N]
    n = w.shape[1]
    return np.ascontiguousarray(w.reshape(CCH, 128, n).transpose(1, 0, 2))


def _consts():
    rt = np.zeros((128, 128), np.float32)
    for i in range(64):
        rt[i + 64, i] = -1.0     # (R @ q)[i] = -q[i+64],  R^T[i+64, i]
        rt[i, i + 64] = 1.0      # (R @ q)[i+64] = q[i],   R^T[i, i+64]
    ident = np.eye(128, dtype=np.float32)
    ones = np.ones((128, 128), np.float32)
    u = np.arange(2 * TQB)[None, :]
    i = np.arange(128)[:, None]
    mask = (u >= i + TQB).astype(np.float32)
    return rt, ident, ones, mask


_NC_CACHE = {}


def kernel(**inputs):
    inp = {k: np.asarray(v, dtype=np.float32) for k, v in inputs.items()}
    q_x, kv_x = inp["q_x"], inp["kv_x"]
    Wq, Wk, Wv, Wo = inp["Wq"], inp["Wk"], inp["Wv"], inp["Wo"]
    bq, bk, bv, bo = inp["bq"], inp["bk"], inp["bv"], inp["bo"]
    with_bias = bool(np.any(bq) or np.any(bk) or np.any(bv))

    if ("nc", with_bias) not in _NC_CACHE:
        _NC_CACHE[("nc", with_bias)] = build_nc(with_bias=with_bias)
    nc = _NC_CACHE[("nc", with_bias)]

    cos, sin = _rope_tables()
    cosT = np.ascontiguousarray(cos.T)   # [HS, T]
    sinT = np.ascontiguousarray(sin.T)
    rt, ident, ones, mask = _consts()

    in_maps = []
    for core in range(NCORES):
        b, g = core // TPG, core % TPG
        m = {
            "xq": _tile_x(q_x[b]),
            "xkv": _tile_x(kv_x[b]),
            "wq": _tile_w(Wq[:, g * HL * HS:(g + 1) * HL * HS]),
            "wkv": _tile_w(np.concatenate(
                [Wk[:, g * KVL * HS:(g + 1) * KVL * HS],
                 Wv[:, g * KVL * HS:(g + 1) * KVL * HS]], axis=1)),
            "wo": np.ascontiguousarray(
                Wo[g * HL * HS:(g + 1) * HL * HS, :]
                .reshape(HL * HS // 128, 128, C).transpose(1, 0, 2)),
            "cosT": cosT, "sinT": sinT,
            "rt": rt, "ident": ident, "ones": ones, "mask": mask,
        }
        if with_bias:
            m["bq"] = bq[g * HL * HS:(g + 1) * HL * HS]
            m["bkv"] = np.concatenate(
                [bk[g * KVL * HS:(g + 1) * KVL * HS],
                 bv[g * KVL * HS:(g + 1) * KVL * HS]])
        in_maps.append(m)

    res = run_bass_kernel_spmd(nc, in_maps, list(range(NCORES)))
    out = np.zeros((B, T, C), np.float32)
    for core in range(NCORES):
        out[core // TPG] += res.results[core]["out"]
    out += bo
    return out


# revision 9
# speedup vs baseline: 1.0286x; 1.0286x over previous
"""GQA attention block (q 32 heads / kv 8 heads, T=2048, C=4096) on 8 trn2
NeuronCores.

Sharding: tensor-parallel over heads x data-parallel over batch.
Core c handles batch b = c//4 and head-group g = c%4 (8 q heads, 2 kv heads).
Each core computes q/k/v projections for its head slice, RoPE, causal
flash-attention, and a row-parallel slice of the output projection; the host
sums the 4 partial outputs per batch.

Device-side layouts keep the head dim (hs) on SBUF partitions:
  Q^T, K^T: [hs=128, tok]  (projections emit transposed directly; RoPE's
                            rotate-half runs as a PSUM->SBUF DMA partition
                            swap + sign folded into the sin table, so the
                            PE stream stays pure matmul)
  V:        [tok, hs]      (projected as V^T, then PE-transposed)
  scores^T: [tk, tq]       (softmax denominators via ones-matmul)
Matmuls run in float32r (fp32 storage, ~1.4e-4 rounding, bf16-rate at
free-dim >= 256).
"""

import os
import sys

for _p in ("/root/.axon_site", "/root/.axon_site/_ro/trn_rl_repo",
           "/root/.axon_site/_ro/pypackages", "/opt/trn_rl_repo", "/opt/pypackages"):
    if os.path.isdir(_p) and _p not in sys.path:
        sys.path.append(_p)

import numpy as np

import concourse.bass as bass
import concourse.tile as tile
from concourse import mybir
from concourse.bass_utils import run_bass_kernel_spmd

F32 = mybir.dt.float32
F32R = mybir.dt.float32r
BF16 = mybir.dt.bfloat16

B, T, C = 2, 2048, 4096
H, KVH, HS = 32, 8, 128
ROPE_BASE = 10000.0

NCORES = 8
TPG = 4               # tensor-parallel groups per batch
HL = H // TPG         # 8 local q heads
KVL = KVH // TPG      # 2 local kv heads
GQ = H // KVH         # 4 q heads per kv head
CCH = C // 128        # 32 contraction chunks
PBLK = 256            # proj token block (free dim)
NPB = T // PBLK       # 8
TQB = 512             # attention tq block
NTQ = T // TQB        # 4
NTK = T // 128        # 16 tk chunks
SCALE = float(1.0 / np.sqrt(HS))

DT = F32R             # matmul operand dtype


def _split1(nc, max_waits=1):
    """Split instructions with >1 sem wait into preceding NOPs (the cayman
    CTRL codegen only accepts one sync-wait command per instruction)."""
    n = 0
    for f in nc.m.functions:
        for bb in f.blocks:
            out = []
            for inst in bb.instructions:
                si = inst.sync_info
                if si is not None and si.on_wait and len(si.on_wait) > max_waits:
                    w = list(si.on_wait)
                    chunks = [w[i:i + max_waits] for i in range(0, len(w), max_waits)]
                    for j, ch in enumerate(chunks[:-1]):
                        out.append(mybir.InstNoOp(
                            name=f"{inst.name}-wsplit{j}", engine=inst.engine,
                            ins=[], outs=[],
                            sync_info=mybir.SyncInfo(on_wait=ch, on_update=[])))
                        n += 1
                    inst.sync_info = mybir.SyncInfo(
                        on_wait=chunks[-1], on_update=list(si.on_update))
                out.append(inst)
            bb.instructions[:] = out
    return n


def build_nc(with_bias=False, split=True):
    nc = bass.Bass("TRN2")
    P = 128

    # --- DRAM parameters (per-core shards, host-pre-tiled layouts) ---
    dp = nc.declare_dram_parameter
    xq_d = dp("xq", [P, NPB, CCH, PBLK], DT, isOutput=False)    # [ki, blk, ko, tw]
    xkv_d = dp("xkv", [P, NPB, CCH, PBLK], DT, isOutput=False)
    wq_d = dp("wq", [P, CCH, HL * HS], DT, isOutput=False)      # [ki, ko, n]
    wkv_d = dp("wkv", [P, CCH, 2 * KVL * HS], DT, isOutput=False)  # [K2 | V2]
    wo_d = dp("wo", [P, HL * HS // P, C], DT, isOutput=False)
    cosT_d = dp("cosT", [HS, T], F32, isOutput=False)
    sinT_d = dp("sinT", [HS, T], F32, isOutput=False)           # sign-folded
    ident_d = dp("ident", [P, P], DT, isOutput=False)
    ones_d = dp("ones", [P, P], DT, isOutput=False)
    mask_d = dp("mask", [P, 2 * TQB], DT, isOutput=False)       # mult. causal
    if with_bias:
        bq_d = dp("bq", [HL * HS], F32, isOutput=False)
        bkv_d = dp("bkv", [2 * KVL * HS], F32, isOutput=False)
    out_d = dp("out", [T, C], F32, isOutput=True)

    qT_d = nc.dram_tensor("qT_i", [HS, HL, T], DT)              # internal

    with tile.TileContext(nc) as tc:
        with (
            tc.tile_pool(name="consts", bufs=1) as consts,
            tc.tile_pool(name="kvres", bufs=1) as kvres,
        ):
            ident_sb = consts.tile([P, P], DT)
            ones_sb = consts.tile([P, P], DT)
            cos_sb = consts.tile([HS, T], F32)
            sin_sb = consts.tile([HS, T], F32)
            nc.sync.dma_start(out=ident_sb, in_=ident_d[:])
            nc.sync.dma_start(out=ones_sb, in_=ones_d[:])
            nc.sync.dma_start(out=cos_sb, in_=cosT_d[:])
            nc.sync.dma_start(out=sin_sb, in_=sinT_d[:])
            if with_bias:
                bq_sb = consts.tile([P, HL], F32)
                bkv_sb = consts.tile([P, 2 * KVL], F32)
                nc.sync.dma_start(out=bq_sb, in_=bq_d[:].rearrange("(h p) -> p h", p=P))
                nc.sync.dma_start(out=bkv_sb, in_=bkv_d[:].rearrange("(h p) -> p h", p=P))

            kT_sb = kvres.tile([HS, KVL, T], DT)
            v_sb = kvres.tile([P, NTK, KVL * HS], DT)

            # ---------- P1 + P2: projections + RoPE ----------
            with (
                tc.tile_pool(name="wpool", bufs=1) as wpool,
                tc.tile_pool(name="xpool", bufs=2) as xpool,
                tc.tile_pool(name="rotp", bufs=4) as rotp,
                tc.tile_pool(name="ptmp", bufs=4) as ptmp,
                tc.tile_pool(name="stage", bufs=4) as stage,
                tc.tile_pool(name="pp_proj", bufs=4, space="PSUM") as pp_proj,
                tc.tile_pool(name="pp_t", bufs=2, space="PSUM") as pp_t,
            ):
                def rope(dst, ps, blk, bias_col=None):
                    """dst = rope(ps + bias) over a [128, PBLK] block.

                    rotate-half = DMA partition swap (sign folded into the
                    host sin table), so nothing here touches the PE.
                    """
                    sl = slice(blk * PBLK, (blk + 1) * PBLK)
                    src = ptmp.tile([P, PBLK], F32, tag="raw")
                    if bias_col is not None:
                        nc.vector.tensor_scalar(
                            out=src, in0=ps, scalar1=bias_col, scalar2=None,
                            op0=mybir.AluOpType.add)
                    else:
                        nc.any.tensor_copy(out=src, in_=ps)
                    rot = rotp.tile([P, PBLK], F32, tag="rot")
                    nc.sync.dma_start(out=rot[0:64, :], in_=src[64:128, :])
                    nc.sync.dma_start(out=rot[64:128, :], in_=src[0:64, :])
                    tmp = ptmp.tile([P, PBLK], F32, tag="ropetmp")
                    nc.vector.tensor_mul(tmp, rot, sin_sb[:, sl])
                    nc.vector.tensor_mul(dst, src, cos_sb[:, sl])
                    nc.vector.tensor_add(dst, dst, tmp)

                # P1: K^T/V^T projections (+ V transpose to natural layout)
                wkv_t = wpool.tile([P, CCH, 2 * KVL * HS], DT, tag="w")
                nc.sync.dma_start(out=wkv_t, in_=wkv_d[:])
                for blk in range(NPB):
                    xkv_t = xpool.tile([P, CCH, PBLK], DT, tag="x")
                    nc.sync.dma_start(out=xkv_t, in_=xkv_d[:, blk])
                    vt_tiles = []
                    for ch in range(2 * KVL):          # k0,k1,v0,v1
                        ps = pp_proj.tile([P, PBLK], F32, tag="proj")
                        for c in range(CCH):
                            nc.tensor.matmul(
                                ps, lhsT=wkv_t[:, c, ch * P:(ch + 1) * P],
                                rhs=xkv_t[:, c, :],
                                start=(c == 0), stop=(c == CCH - 1))
                        if ch < KVL:                   # K head: RoPE -> kT_sb
                            dst = kT_sb[:, ch, blk * PBLK:(blk + 1) * PBLK]
                            rope(dst, ps, blk,
                                 bkv_sb[:, ch:ch + 1] if with_bias else None)
                        else:                          # V head: evict, defer T
                            vt = stage.tile([P, PBLK], DT, tag="vt")
                            if with_bias:
                                nc.vector.tensor_scalar(
                                    out=vt, in0=ps,
                                    scalar1=bkv_sb[:, KVL + (ch - KVL):
                                                   KVL + (ch - KVL) + 1],
                                    scalar2=None, op0=mybir.AluOpType.add)
                            else:
                                nc.any.tensor_copy(out=vt, in_=ps)
                            vt_tiles.append((ch - KVL, vt))
                    for kv, vt in vt_tiles:            # deferred PE transposes
                        for sub in range(PBLK // P):
                            pt = pp_t.tile([P, P], DT, tag="tp")
                            nc.tensor.transpose(
                                pt, vt[:, sub * P:(sub + 1) * P], ident_sb)
                            nc.any.tensor_copy(
                                out=v_sb[:, 2 * blk + sub,
                                         kv * HS:(kv + 1) * HS],
                                in_=pt)

                # P2: Q^T projection (2 weight halves)
                for half in range(2):
                    wq_t = wpool.tile([P, CCH, HL * HS // 2], DT, tag="w")
                    nc.sync.dma_start(
                        out=wq_t, in_=wq_d[:, :, half * 512:(half + 1) * 512])
                    for blk in range(NPB):
                        xq_t = xpool.tile([P, CCH, PBLK], DT, tag="x")
                        nc.sync.dma_start(out=xq_t, in_=xq_d[:, blk])
                        for hb in range(HL // 2):
                            h = half * (HL // 2) + hb
                            ps = pp_proj.tile([P, PBLK], F32, tag="proj")
                            for c in range(CCH):
                                nc.tensor.matmul(
                                    ps, lhsT=wq_t[:, c, hb * P:(hb + 1) * P],
                                    rhs=xq_t[:, c, :],
                                    start=(c == 0), stop=(c == CCH - 1))
                            qro = stage.tile([P, PBLK], DT, tag="qro")
                            rope(qro, ps, blk,
                                 bq_sb[:, h:h + 1] if with_bias else None)
                            nc.sync.dma_start(
                                out=qT_d[:, h, blk * PBLK:(blk + 1) * PBLK],
                                in_=qro)

            # ---------- P3: attention, P4: output projection ----------
            with (
                tc.tile_pool(name="ybuf", bufs=1) as ybuf,
                tc.tile_pool(name="qbuf", bufs=2) as qbuf,
                tc.tile_pool(name="pbuf", bufs=4) as pbuf,
                tc.tile_pool(name="rbuf", bufs=2) as rbuf,
                tc.tile_pool(name="mbuf", bufs=1) as mbuf,
                tc.tile_pool(name="pp_s", bufs=2, space="PSUM") as pp_s,
                tc.tile_pool(name="pp_acc", bufs=2, space="PSUM") as pp_acc,
            ):
                mask_sb = mbuf.tile([P, 2 * TQB], DT)
                nc.sync.dma_start(out=mask_sb, in_=mask_d[:])
                yT_sb = ybuf.tile([HS, HL, T], DT)
                for h in range(HL):
                    kv = h // GQ
                    for j in range(NTQ):
                        qb = qbuf.tile([HS, TQB], DT, tag="qb")
                        nc.sync.dma_start(
                            out=qb, in_=qT_d[:, h, j * TQB:(j + 1) * TQB])
                        ps_y = pp_acc.tile([P, TQB], F32, tag="accy")
                        ps_sum = pp_acc.tile([P, TQB], F32, tag="accs")
                        nchunks = (j + 1) * (TQB // P)

                        def flush(pend, last):
                            pT0, a0 = pend
                            nc.tensor.matmul(
                                ps_y, lhsT=v_sb[:, a0, kv * HS:(kv + 1) * HS],
                                rhs=pT0, start=(a0 == 0), stop=last)
                            nc.tensor.matmul(
                                ps_sum, lhsT=ones_sb, rhs=pT0,
                                start=(a0 == 0), stop=last)

                        pend = None
                        for a in range(nchunks):
                            ps_s = pp_s.tile([P, TQB], F32, tag="s")
                            nc.tensor.matmul(
                                ps_s, lhsT=kT_sb[:, kv, a * P:(a + 1) * P],
                                rhs=qb, start=True, stop=True)
                            if pend is not None:
                                flush(pend, last=False)
                            pT = pbuf.tile([P, TQB], DT, tag="pT")
                            nc.scalar.activation(
                                out=pT, in_=ps_s,
                                func=mybir.ActivationFunctionType.Exp,
                                scale=SCALE)
                            m = a - j * (TQB // P)
                            if m >= 0:                 # diagonal chunk
                                nc.vector.tensor_mul(
                                    pT, pT,
                                    mask_sb[:, TQB - m * P: 2 * TQB - m * P])
                            pend = (pT, a)
                        flush(pend, last=True)
                        rec = rbuf.tile([P, TQB], F32, tag="rec")
                        nc.vector.reciprocal(out=rec, in_=ps_sum)
                        nc.vector.tensor_mul(
                            yT_sb[:, h, j * TQB:(j + 1) * TQB], ps_y, rec)

                # P4: out = yT.T @ Wo (row-parallel partial)
                with (
                    tc.tile_pool(name="wobuf", bufs=2) as wobuf,
                    tc.tile_pool(name="obuf", bufs=3) as obuf,
                    tc.tile_pool(name="pp_o", bufs=2, space="PSUM") as pp_o,
                ):
                    NCH = HL * HS // P                 # 8
                    for o in range(C // TQB):          # 8 output chunks
                        wo_t = wobuf.tile([P, NCH, TQB], DT, tag="wo")
                        nc.sync.dma_start(
                            out=wo_t, in_=wo_d[:, :, o * TQB:(o + 1) * TQB])
                        for i in range(NTK):           # 16 token chunks
                            ps = pp_o.tile([P, TQB], F32, tag="o")
                            for chl in range(NCH):
                                nc.tensor.matmul(
                                    ps, lhsT=yT_sb[:, chl, i * P:(i + 1) * P],
                                    rhs=wo_t[:, chl, :],
                                    start=(chl == 0), stop=(chl == NCH - 1))
                            ot = obuf.tile([P, TQB], F32, tag="ot")
                            nc.any.tensor_copy(out=ot, in_=ps)
                            nc.sync.dma_start(
                                out=out_d[i * P:(i + 1) * P,
                                          o * TQB:(o + 1) * TQB],
                                in_=ot)

    if split:
        _split1(nc)
    return nc


def _rope_tables():
    inv_freq = (1.0 / (np.float32(ROPE_BASE) **
                       (np.arange(0, HS, 2, dtype=np.float32) / np.float32(HS))))
    pos = np.arange(T, dtype=np.float32)
    ang = pos[:, None] * inv_freq[None, :]
    ang = np.concatenate([ang, ang], axis=-1).astype(np.float32)  # [T, HS]
    return np.cos(ang).astype(np.float32), np.sin(ang).astype(np.float32)


def _tile_x(x):
    # [T, C] -> [ki=128, blk, ko, tw]  (x^T tiled for contraction-major DMA)
    return np.ascontiguousarray(
        x.reshape(NPB, PBLK, CCH, 128).transpose(3, 0, 2, 1))


def _tile_w(w):
    # [C, N] -> [ki=128, ko, N]
    n = w.shape[1]
    return np.ascontiguousarray(w.reshape(CCH, 128, n).transpose(1, 0, 2))


def _consts():
    ident = np.eye(128, dtype=np.float32)
    ones = np.ones((128, 128), np.float32)
    u = np.arange(2 * TQB)[None, :]
    i = np.arange(128)[:, None]
    mask = (u >= i + TQB).astype(np.float32)
    return ident, ones, mask


_NC_CACHE = {}


def make_in_maps(inp, with_bias):
    q_x, kv_x = inp["q_x"], inp["kv_x"]
    Wq, Wk, Wv, Wo = inp["Wq"], inp["Wk"], inp["Wv"], inp["Wo"]
    cos, sin = _rope_tables()
    cosT = np.ascontiguousarray(cos.T)            # [HS, T]
    sinT = np.ascontiguousarray(sin.T).copy()
    sinT[:64, :] *= -1.0                          # sign of rotate-half
    ident, ones, mask = _consts()
    in_maps = []
    for core in range(NCORES):
        b, g = core // TPG, core % TPG
        m = {
            "xq": _tile_x(q_x[b]),
            "xkv": _tile_x(kv_x[b]),
            "wq": _tile_w(Wq[:, g * HL * HS:(g + 1) * HL * HS]),
            "wkv": _tile_w(np.concatenate(
                [Wk[:, g * KVL * HS:(g + 1) * KVL * HS],
                 Wv[:, g * KVL * HS:(g + 1) * KVL * HS]], axis=1)),
            "wo": np.ascontiguousarray(
                Wo[g * HL * HS:(g + 1) * HL * HS, :]
                .reshape(HL * HS // 128, 128, C).transpose(1, 0, 2)),
            "cosT": cosT, "sinT": sinT,
            "ident": ident, "ones": ones, "mask": mask,
        }
        if with_bias:
            m["bq"] = np.ascontiguousarray(inp["bq"][g * HL * HS:(g + 1) * HL * HS])
            m["bkv"] = np.concatenate(
                [inp["bk"][g * KVL * HS:(g + 1) * KVL * HS],
                 inp["bv"][g * KVL * HS:(g + 1) * KVL * HS]])
        in_maps.append(m)
    return in_maps


def kernel(**inputs):
    inp = {k: np.asarray(v, dtype=np.float32) for k, v in inputs.items()}
    with_bias = bool(np.any(inp["bq"]) or np.any(inp["bk"]) or np.any(inp["bv"]))

    if ("nc", with_bias) not in _NC_CACHE:
        _NC_CACHE[("nc", with_bias)] = build_nc(with_bias=with_bias)
    nc = _NC_CACHE[("nc", with_bias)]

    in_maps = make_in_maps(inp, with_bias)
    res = run_bass_kernel_spmd(nc, in_maps, list(range(NCORES)))
    out = np.zeros((B, T, C), np.float32)
    for core in range(NCORES):
        out[core // TPG] += res.results[core]["out"]
    out += inp["bo"]
    return out
